# revision 13
# baseline (speedup 1.0000x reference)
"""CERNN model kernel for 8 trn2 NeuronCores.

Strategy: 8-way tensor parallel over the 4096 RNN units (512 per core).
Recurrent weights live in SBUF as a bf16 hi+lo split; each scan step does
96 accumulating matmuls (3 split terms x 32 K-tiles) with the hidden state
(transposed, [unit, batch]) as the 32-col stationary operand and the weight
tiles as the 512-wide moving operand.  Per-step hidden slices are exchanged
with an AllGather (bf16 hi/lo pair) inside hardware For_i loops.
"""
import numpy as np
import concourse.bass as bass
import concourse.bacc as bacc
import concourse.tile as tile
import concourse.mybir as mybir
from concourse import bass_utils

F32 = mybir.dt.float32
BF16 = mybir.dt.bfloat16
AX = mybir.AxisListType
OP = mybir.AluOpType
ACT = mybir.ActivationFunctionType

H = 4096
HC = 512          # units per core
B = 32
T = 256
NI = 85
NO = 33
NMOT = 64
QG = 32           # global K tiles (128 units each)
QC = 4            # my K tiles
NCORES = 8
DECAY = 0.2
ZTHR = 0.01
DA_GAIN = 0.2
FPI = 100

_CACHE = {}
SKIP_AG = False


def _input_mask_np():
    m = np.zeros((H, NI), np.float32)
    m[64:128, 1:3] = 1.0
    m[0:64, 3:5] = 1.0
    m[0:64, 0] = 1.0
    m[:, 5:NI - 1] = 1.0
    return m


def _build(t_steps, fp_iters):
    nc = bacc.Bacc("TRN2", target_bir_lowering=False, debug=False,
                   num_devices=NCORES)
    RG = [list(range(NCORES))]

    # ---- per-core external inputs (host pre-sharded/laid out) ----
    wt_d = nc.dram_tensor("wt", [H, HC], F32, kind="ExternalInput")      # W slice, transposed
    mt_d = nc.dram_tensor("mt", [H, HC], F32, kind="ExternalInput")      # intra-area mask slice, transposed
    xT_d = nc.dram_tensor("xT", [NI, t_steps * B], F32, kind="ExternalInput")  # x transposed, t-major
    wihT_d = nc.dram_tensor("wihT", [NI, HC], F32, kind="ExternalInput")
    mihT_d = nc.dram_tensor("mihT", [NI, HC], F32, kind="ExternalInput")
    x01_d = nc.dram_tensor("x01", [NI, 2], F32, kind="ExternalInput")
    biasf_d = nc.dram_tensor("biasf", [128, QC], F32, kind="ExternalInput")
    d1x_d = nc.dram_tensor("d1x", [128, QC * B], F32, kind="ExternalInput")
    idt_d = nc.dram_tensor("idt", [128, 128], F32, kind="ExternalInput")
    woT_d = nc.dram_tensor("woT", [NMOT, NO], F32, kind="ExternalInput")
    # ---- outputs ----
    hid_d = nc.dram_tensor("hid", [t_steps, 128, QC * B], F32, kind="ExternalOutput")
    mot_d = nc.dram_tensor("mot", [t_steps, NO, B], F32, kind="ExternalOutput")
    h0dbg_d = nc.dram_tensor("h0dbg", [128, QG], F32, kind="ExternalOutput")

    with tile.TileContext(nc) as tc:
        with tc.tile_pool(name="persist", bufs=1) as pp, \
             tc.tile_pool(name="wpool", bufs=1) as wp, \
             tc.tile_pool(name="work", bufs=3) as wk, \
             tc.tile_pool(name="ps", bufs=2, space="PSUM") as ps, \
             tc.tile_pool(name="dram", bufs=1, space="DRAM") as dp:

            # ---------- constants ----------
            idt = pp.tile([128, 128], F32, tag="idt")
            nc.sync.dma_start(idt[:], idt_d.ap())
            biasf = pp.tile([128, QC], F32, tag="biasf")
            nc.sync.dma_start(biasf[:], biasf_d.ap())
            d1x = pp.tile([128, QC * B], F32, tag="d1x")
            nc.sync.dma_start(d1x[:], d1x_d.ap())
            m02x = pp.tile([128, QC * B], F32, tag="m02x")
            m08x = pp.tile([128, QC * B], F32, tag="m08x")
            # mult = 1 + 0.2*2*(d1-0.5); 0.2*mult = 0.08*d1+0.16 ; 0.8*mult = 0.32*d1+0.64
            nc.scalar.activation(m02x[:], d1x[:], ACT.Copy, bias=0.16, scale=0.08)
            nc.scalar.activation(m08x[:], d1x[:], ACT.Copy, bias=0.64, scale=0.32)
            ones32 = pp.tile([128, B], F32, tag="ones32")
            nc.vector.memset(ones32[:], 1.0)
            onescol = pp.tile([128, 1], BF16, tag="onescol")
            nc.vector.memset(onescol[:], 1.0)
            onesrow = pp.tile([1, 128], BF16, tag="onesrow")
            nc.vector.memset(onesrow[:], 1.0)
            woT = pp.tile([NMOT, NO], F32, tag="woT")
            nc.sync.dma_start(woT[:], woT_d.ap())

            # ---------- masked input weights ----------
            wihmT = pp.tile([NI, HC], F32, tag="wihmT")
            wtmp = wk.tile([NI, HC], F32, tag="wihtmp")
            nc.sync.dma_start(wihmT[:], wihT_d.ap())
            nc.sync.dma_start(wtmp[:], mihT_d.ap())
            nc.vector.tensor_tensor(wihmT[:], wihmT[:], wtmp[:], op=OP.mult)

            # ---------- recurrent weight prep: mask + bf16 hi/lo split ----------
            whi = [wp.tile([128, HC], BF16, tag=f"whi{q}", name=f"whi{q}") for q in range(QG)]
            wlo = [wp.tile([128, HC], BF16, tag=f"wlo{q}", name=f"wlo{q}") for q in range(QG)]
            for q in range(QG):
                wq = wk.tile([128, HC], F32, tag="wq")
                mq = wk.tile([128, HC], F32, tag="mq")
                nc.sync.dma_start(wq[:], wt_d.ap()[q * 128:(q + 1) * 128, :])
                nc.sync.dma_start(mq[:], mt_d.ap()[q * 128:(q + 1) * 128, :])
                aq = wk.tile([128, HC], F32, tag="aq")
                nc.scalar.activation(aq[:], wq[:], ACT.Abs)
                nc.vector.tensor_scalar(aq[:], aq[:], ZTHR, None, op0=OP.is_gt)
                nc.vector.tensor_tensor(wq[:], wq[:], mq[:], op=OP.mult)
                nc.vector.tensor_tensor(wq[:], wq[:], aq[:], op=OP.mult)
                nc.vector.tensor_copy(whi[q][:], wq[:])
                nc.vector.tensor_tensor(wlo[q][:], wq[:], whi[q][:], op=OP.subtract)

            # ---------- input projection precompute ----------
            # inp_dram[t, p, q*32+b] = (x_t @ wih_masked.T).T + bias  (bias folded)
            inp_dram = dp.tile([t_steps, 128, QC * B], F32, tag="inpd")
            n_chunk = (t_steps * B) // 512
            for q in range(QC):
                for ci in range(n_chunk):
                    xc = wk.tile([NI, 512], F32, tag="xc")
                    nc.sync.dma_start(xc[:], xT_d.ap()[:, ci * 512:(ci + 1) * 512])
                    pc = ps.tile([128, 512], F32, tag="psT")
                    nc.tensor.matmul(pc[:], wihmT[:, q * 128:(q + 1) * 128], xc[:],
                                     start=True, stop=True)
                    sc = wk.tile([128, 512], F32, tag="sc")
                    nc.scalar.activation(sc[:], pc[:], ACT.Identity,
                                         bias=biasf[:, q:q + 1], scale=1.0)
                    tt0 = ci * (512 // B)
                    dst = inp_dram[tt0:tt0 + 16, :, q * B:(q + 1) * B].transpose([1, 0, 2])
                    nc.sync.dma_start(dst, sc[:].rearrange("p (t b) -> p t b", b=B))

            # fp-phase inputs i0/i1 (+bias), transposed layout [128, QC, 2]
            i01b = pp.tile([128, QC, 2], F32, tag="i01b")
            x01 = pp.tile([NI, 2], F32, tag="x01")
            nc.sync.dma_start(x01[:], x01_d.ap())
            for q in range(QC):
                pq = ps.tile([128, 2], F32, tag="psT")
                nc.tensor.matmul(pq[:], wihmT[:, q * 128:(q + 1) * 128], x01[:],
                                 start=True, stop=True)
                nc.scalar.activation(i01b[:, q, :], pq[:], ACT.Identity,
                                     bias=biasf[:, q:q + 1], scale=1.0)

            # ---------- fp-phase state ----------
            h0f = pp.tile([128, QG], F32, tag="h0f")        # full h0 (transposed fold)
            h0my = pp.tile([128, QC], F32, tag="h0my")      # my slice fp32
            h0hi = pp.tile([128, QG], BF16, tag="h0hi")
            h0lo = pp.tile([128, QG], BF16, tag="h0lo")
            stab = pp.tile([1, 1], F32, tag="stab")
            done = pp.tile([1, 1], F32, tag="done")
            gate = pp.tile([1, 1], F32, tag="gate")
            gateb = pp.tile([128, 1], F32, tag="gateb")
            for t_ in (h0f, h0my, stab, done):
                nc.vector.memset(t_[:], 0.0)
            for t_ in (h0hi, h0lo):
                nc.vector.memset(t_[:], 0.0)

            # fp bounce + gathered buffers (two cells: u and h)
            fb_u = dp.tile([128, 2 * QC], BF16, tag="fb_u")
            fg_u = dp.tile([128 * NCORES, 2 * QC], BF16, tag="fg_u")
            fb_h = dp.tile([128, 2 * QC], BF16, tag="fb_h")
            fg_h = dp.tile([128 * NCORES, 2 * QC], BF16, tag="fg_h")
            uhi = pp.tile([128, QG], BF16, tag="uhi")
            ulo = pp.tile([128, QG], BF16, tag="ulo")
            hhi = pp.tile([128, QG], BF16, tag="hhi")
            hlo = pp.tile([128, QG], BF16, tag="hlo")
            umy = pp.tile([128, QC], F32, tag="umy")
            hmy = pp.tile([128, QC], F32, tag="hmy")
            hful = pp.tile([128, QG], F32, tag="hful")

            def fp_cell(src_hi, src_lo, decay_src, inp_col, out_my, bounce, gathered,
                        out_hi, out_lo):
                pA = ps.tile([1, HC], F32, tag="pmm")
                for q in range(QG):
                    nc.tensor.matmul(pA[:], src_hi[:, q:q + 1], whi[q][:],
                                     start=(q == 0), stop=False)
                    nc.tensor.matmul(pA[:], src_lo[:, q:q + 1], whi[q][:],
                                     start=False, stop=False)
                    nc.tensor.matmul(pA[:], src_hi[:, q:q + 1], wlo[q][:],
                                     start=False, stop=(q == QG - 1))
                praw = wk.tile([1, HC], F32, tag="praw")
                nc.vector.tensor_copy(praw[:], pA[:])
                pT = ps.tile([128, QC], F32, tag="psT2")
                for k in range(QC):
                    nc.tensor.transpose(pT[:, k:k + 1], praw[0:1, k * 128:(k + 1) * 128],
                                        idt[0:1, 0:1])
                t1 = wk.tile([128, QC], F32, tag="fpt1")
                nc.vector.tensor_tensor(t1[:], pT[:], inp_col, op=OP.add)
                nc.scalar.activation(t1[:], t1[:], ACT.Relu)
                nc.vector.tensor_tensor(t1[:], t1[:], m02x[:].rearrange(
                    "p (q b) -> p q b", b=B)[:, :, 0], op=OP.mult)
                t4 = wk.tile([128, QC], F32, tag="fpt4")
                nc.vector.tensor_tensor(t4[:], decay_src[:], m08x[:].rearrange(
                    "p (q b) -> p q b", b=B)[:, :, 0], op=OP.mult)
                nc.vector.tensor_tensor(out_my[:], t1[:], t4[:], op=OP.add)
                # hi/lo split of my slice, packed for the allgather
                hilo = wk.tile([128, 2 * QC], BF16, tag="fphilo")
                nc.vector.tensor_copy(hilo[:, 0:QC], out_my[:])
                nc.vector.tensor_tensor(hilo[:, QC:2 * QC], out_my[:], hilo[:, 0:QC],
                                        op=OP.subtract)
                nc.sync.dma_start(bounce[:], hilo[:])
                nc.gpsimd.collective_compute(
                    "AllGather", OP.bypass, replica_groups=RG,
                    ins=[bounce[:].opt()], outs=[gathered[:].opt()])
                src = gathered[:].rearrange("(r p) c -> p r c", p=128)
                nc.sync.dma_start(
                    out_hi[:].rearrange("p (r q) -> p r q", r=NCORES),
                    src[:, :, 0:QC])
                nc.sync.dma_start(
                    out_lo[:].rearrange("p (r q) -> p r q", r=NCORES),
                    src[:, :, QC:2 * QC])

            for fi in range(fp_iters):
                fp_cell(h0hi, h0lo, h0my, i01b[:, :, 0], umy, fb_u, fg_u, uhi, ulo)
                fp_cell(uhi, ulo, umy, i01b[:, :, 1], hmy, fb_h, fg_h, hhi, hlo)
                # convergence bookkeeping (reference fp_body semantics)
                nc.vector.tensor_tensor(hful[:], hhi[:], hlo[:], op=OP.add)
                ad = wk.tile([128, QG], F32, tag="cvad")
                nc.vector.tensor_tensor(ad[:], hful[:], h0f[:], op=OP.subtract)
                nc.scalar.activation(ad[:], ad[:], ACT.Abs)
                thr = wk.tile([128, QG], F32, tag="cvthr")
                nc.scalar.activation(thr[:], h0f[:], ACT.Abs)
                nc.scalar.activation(thr[:], thr[:], ACT.Copy, bias=0.1, scale=1e-5)
                nc.vector.tensor_tensor(ad[:], ad[:], thr[:], op=OP.is_gt)
                exr = wk.tile([128, 1], F32, tag="cvexr")
                nc.vector.tensor_reduce(exr[:], ad[:], axis=AX.X, op=OP.add)
                exb = wk.tile([128, 1], BF16, tag="cvexb")
                nc.vector.tensor_copy(exb[:], exr[:])
                pcnt = ps.tile([1, 1], F32, tag="pcnt")
                nc.tensor.matmul(pcnt[:], exb[:], onescol[:], start=True, stop=True)
                cl = wk.tile([1, 1], F32, tag="cvcl")
                nc.vector.tensor_scalar(cl[:], pcnt[:], 0.5, None, op0=OP.is_lt)
                nc.vector.tensor_scalar(stab[:], stab[:], 1.0, None, op0=OP.add)
                nc.vector.tensor_tensor(stab[:], stab[:], cl[:], op=OP.mult)
                rch = wk.tile([1, 1], F32, tag="cvrch")
                nc.vector.tensor_scalar(rch[:], stab[:], 3.5, None, op0=OP.is_ge)
                nc.vector.tensor_tensor(done[:], done[:], rch[:], op=OP.max)
                nc.scalar.activation(gate[:], done[:], ACT.Copy, bias=1.0, scale=-1.0)
                gbf = wk.tile([1, 1], BF16, tag="cvgbf")
                nc.vector.tensor_copy(gbf[:], gate[:])
                pgb = ps.tile([128, 1], F32, tag="pcnt")
                nc.tensor.matmul(pgb[:], onesrow[:], gbf[:], start=True, stop=True)
                nc.vector.tensor_copy(gateb[:], pgb[:])
                # h0 <- h0 + gate*(h - h0)  (full, my slice, and bf16 splits)
                dd = wk.tile([128, QG], F32, tag="cvdd")
                nc.vector.tensor_tensor(dd[:], hful[:], h0f[:], op=OP.subtract)
                nc.vector.tensor_scalar(dd[:], dd[:], gateb[:, 0:1], None, op0=OP.mult)
                nc.vector.tensor_tensor(h0f[:], h0f[:], dd[:], op=OP.add)
                dm = wk.tile([128, QC], F32, tag="cvdm")
                nc.vector.tensor_tensor(dm[:], hmy[:], h0my[:], op=OP.subtract)
                nc.vector.tensor_scalar(dm[:], dm[:], gateb[:, 0:1], None, op0=OP.mult)
                nc.vector.tensor_tensor(h0my[:], h0my[:], dm[:], op=OP.add)
                nc.vector.tensor_copy(h0hi[:], h0f[:])
                nc.vector.tensor_tensor(h0lo[:], h0f[:], h0hi[:], op=OP.subtract)

            nc.sync.dma_start(h0dbg_d.ap(), h0f[:])

            # ---------- broadcast h0 across batch into main-scan state ----------
            hThi = [pp.tile([128, QG * B], BF16, tag=f"hThi{i}", name=f"hThi{i}") for i in range(2)]
            hTlo = [pp.tile([128, QG * B], BF16, tag=f"hTlo{i}", name=f"hTlo{i}") for i in range(2)]
            hmyf = [pp.tile([128, QC * B], F32, tag=f"hmyf{i}", name=f"hmyf{i}") for i in range(2)]
            h0lo32 = pp.tile([128, QG], F32, tag="h0lo32")
            nc.vector.tensor_copy(h0lo32[:], h0lo[:])
            for q in range(QG):
                nc.vector.tensor_scalar(
                    hThi[0][:, q * B:(q + 1) * B], ones32[:], h0f[:, q:q + 1], None,
                    op0=OP.mult)
                nc.vector.tensor_scalar(
                    hTlo[0][:, q * B:(q + 1) * B], ones32[:], h0lo32[:, q:q + 1], None,
                    op0=OP.mult)
            for q in range(QC):
                nc.vector.tensor_scalar(
                    hmyf[0][:, q * B:(q + 1) * B], ones32[:], h0my[:, q:q + 1], None,
                    op0=OP.mult)

            # main-scan bounce buffers (parity A/B)
            mb = [dp.tile([128, 2 * QC * B], BF16, tag=f"mb{i}", name=f"mb{i}") for i in range(2)]
            mg = [dp.tile([128 * NCORES, 2 * QC * B], BF16, tag=f"mg{i}",
                          name=f"mg{i}") for i in range(2)]

            def scan_step(tslice, src, dst):
                """tslice: dram AP index for this step (1-sized). src/dst: parity."""
                pM = ps.tile([B, HC], F32, tag="pmm")
                s_hi, s_lo = hThi[src], hTlo[src]
                for q in range(QG):
                    nc.tensor.matmul(pM[:], s_hi[:, q * B:(q + 1) * B], whi[q][:],
                                     start=(q == 0), stop=False)
                    nc.tensor.matmul(pM[:], s_lo[:, q * B:(q + 1) * B], whi[q][:],
                                     start=False, stop=False)
                    nc.tensor.matmul(pM[:], s_hi[:, q * B:(q + 1) * B], wlo[q][:],
                                     start=False, stop=(q == QG - 1))
                praw = wk.tile([B, HC], F32, tag="mpraw")
                nc.vector.tensor_copy(praw[:], pM[:])
                pT = ps.tile([128, QC * B], F32, tag="psT")
                for k in range(QC):
                    nc.tensor.transpose(pT[:, k * B:(k + 1) * B],
                                        praw[0:B, k * 128:(k + 1) * 128],
                                        idt[0:B, 0:B])
                inpt = wk.tile([128, QC * B], F32, tag="minp")
                nc.sync.dma_start(inpt[:], inp_dram[tslice, :, :])
                t1 = wk.tile([128, QC * B], F32, tag="mt1")
                nc.vector.tensor_tensor(t1[:], pT[:], inpt[:], op=OP.add)
                nc.scalar.activation(t1[:], t1[:], ACT.Relu)
                nc.vector.tensor_tensor(t1[:], t1[:], m02x[:], op=OP.mult)
                t4 = wk.tile([128, QC * B], F32, tag="mt4")
                nc.vector.tensor_tensor(t4[:], hmyf[src][:], m08x[:], op=OP.mult)
                nc.vector.tensor_tensor(hmyf[dst][:], t1[:], t4[:], op=OP.add)
                # outputs
                nc.sync.dma_start(hid_d.ap()[tslice, :, :], hmyf[dst][:])
                pm = ps.tile([NO, B], F32, tag="pcnt")
                nc.tensor.matmul(pm[:], woT[:], hmyf[dst][0:NMOT, B:2 * B],
                                 start=True, stop=True)
                som = wk.tile([NO, B], F32, tag="msom")
                nc.vector.tensor_copy(som[:], pm[:])
                nc.sync.dma_start(mot_d.ap()[tslice, :, :], som[:])
                # hi/lo split + exchange
                hilo = wk.tile([128, 2 * QC * B], BF16, tag="mhilo")
                nc.vector.tensor_copy(hilo[:, 0:QC * B], hmyf[dst][:])
                nc.vector.tensor_tensor(hilo[:, QC * B:], hmyf[dst][:],
                                        hilo[:, 0:QC * B], op=OP.subtract)
                nc.sync.dma_start(mb[dst][:], hilo[:])
                if not SKIP_AG:
                    nc.gpsimd.collective_compute(
                        "AllGather", OP.bypass, replica_groups=RG,
                        ins=[mb[dst][:].opt()], outs=[mg[dst][:].opt()])
                    srcg = mg[dst][:].rearrange("(r p) c -> p r c", p=128)
                    nc.sync.dma_start(
                        hThi[dst][:].rearrange("p (r q) -> p r q", r=NCORES),
                        srcg[:, :, 0:QC * B])
                    nc.sync.dma_start(
                        hTlo[dst][:].rearrange("p (r q) -> p r q", r=NCORES),
                        srcg[:, :, QC * B:])
                else:
                    srcg = mb[dst][:].rearrange("p (two c) -> p two c", two=2)
                    for r in range(NCORES):
                        nc.sync.dma_start(
                            hThi[dst][:, r * QC * B:(r + 1) * QC * B], srcg[:, 0, :])
                        nc.sync.dma_start(
                            hTlo[dst][:, r * QC * B:(r + 1) * QC * B], srcg[:, 1, :])

            for mi in range(t_steps // 2):
                scan_step(slice(mi * 2, mi * 2 + 1), 0, 1)
                scan_step(slice(mi * 2 + 1, mi * 2 + 2), 1, 0)

    nc.compile()
    return nc


def _prep_inputs(x, weight_ih, weight_hh, bias, w_out, d1_map, intra_area_mask,
                 t_steps):
    x = np.ascontiguousarray(x, np.float32)
    mask_ih = _input_mask_np()
    xT = np.ascontiguousarray(
        x[:t_steps].transpose(2, 0, 1).reshape(NI, t_steps * B))
    x01 = np.ascontiguousarray(x[0:2, 0, :].T)  # [85, 2]
    idt = np.eye(128, dtype=np.float32)
    woT = np.ascontiguousarray(w_out.T, np.float32)  # [64, 33]
    in_maps = []
    for c in range(NCORES):
        sl = slice(c * HC, (c + 1) * HC)
        wt = np.ascontiguousarray(weight_hh[sl, :].T, np.float32)
        mt = np.ascontiguousarray(intra_area_mask[sl, :].T, np.float32)
        wihT = np.ascontiguousarray(weight_ih[sl, :].T, np.float32)
        mihT = np.ascontiguousarray(mask_ih[sl, :].T, np.float32)
        biasf = np.ascontiguousarray(
            bias[sl].reshape(QC, 128).T, np.float32)          # [128, QC]
        d1f = d1_map[sl].reshape(QC, 128).T                    # [128, QC]
        d1x = np.ascontiguousarray(
            np.repeat(d1f[:, :, None], B, axis=2).reshape(128, QC * B), np.float32)
        in_maps.append({
            "wt": wt, "mt": mt, "xT": xT, "wihT": wihT, "mihT": mihT,
            "x01": x01, "biasf": biasf, "d1x": d1x, "idt": idt, "woT": woT,
        })
    return in_maps


def kernel(x, weight_ih, weight_hh, bias, w_out, d1_map, intra_area_mask,
           t_steps=T, fp_iters=FPI):
    key = (t_steps, fp_iters)
    if key not in _CACHE:
        _CACHE[key] = _build(t_steps, fp_iters)
    nc = _CACHE[key]
    in_maps = _prep_inputs(x, weight_ih, weight_hh, bias, w_out, d1_map,
                           intra_area_mask, t_steps)
    res = bass_utils.run_bass_kernel_spmd(nc, in_maps, core_ids=list(range(NCORES)))
    outs = res.results
    # assemble hidden [T, B, H] from per-core [T, 128, QC*B]
    hid = np.empty((t_steps, B, H), np.float32)
    for c in range(NCORES):
        a = outs[c]["hid"].reshape(t_steps, 128, QC, B)
        hid[:, :, c * HC:(c + 1) * HC] = a.transpose(0, 3, 2, 1).reshape(
            t_steps, B, HC)
    out = np.ascontiguousarray(outs[0]["mot"].transpose(0, 2, 1))
    return out, hid
